# revision 13
# baseline (speedup 1.0000x reference)
"""Trainium2 Bass kernel for a supervised-contrastive-style loss.

Reference computation (see problem statement):
  - dropout(p=0.5, scale 2, jax key 42) on gathered class-2 rows, concat -> feats [N2, D]
  - fn = feats / max(||feats||, 1e-8);  sim = fn @ fn.T / T
  - denom_i = sum_j exp(sim_ij) * [labs_i == labs_j]
  - loss = -mean(sim_ii - log denom_i)

Strategy:
  * Host: mirror the reference prologue (dropout/concat/normalize) op-for-op on
    the default jax backend (bit-identical PRNG + fn), then sort rows by class.
    The label mask becomes block-diagonal, so the device only computes
    same-class row x col tiles (~46% of the full N2^2 work).
  * Symmetry: within a class block sim is symmetric, so only upper-triangle
    (row-tile[128] x col-panel[<=512]) tiles are computed.  Each tile yields a
    row-sum (ScalarE exp activation with accum_out) and a column-sum.  Rows
    are dealt to the 8 cores with a stride-8 "comb" (core k owns tiles k,
    k+8, ...) so every core runs the *same* staircase program; the few
    below-diagonal jobs this over-approximates are ignored on the host.
  * fp8: fn is scaled by S=16 and quantized to fp8e4 on host.  Main matmuls
    run in DoubleRow perf mode (K=256 per matmul, ~1.6x PE throughput) and
    panel DMA traffic drops 4x vs f32 (one contiguous DMA per panel group).
  * Activation pairing: per-instruction overhead on ScalarE (~300ns) is
    comparable to the 512-element streaming time, so full-width panels are
    processed in PAIRS (odd,even): one [128, 2*512] exp activation per row
    slot covers two matmul jobs.  The diagonal panel of row slot j is always
    even (p=2j), so it is never inside a pair and keeps its own activation.
    The paired accum_out (row-sum over both panels) is exactly what the host
    needs, because both panels of a pair are at-or-above the diagonal for
    every core.
  * Column sums: exp tiles are written in bf16; a ones-vector matmul per job
    computes the column sum.  Four jobs' colsums are packed into one PSUM
    tile via tensor-engine column tiling (tile_position=(0,32g)); the pack is
    emitted several jobs late so the PE never waits on ScalarE.  The PSUM
    tile is drained to a staging SBUF tile by the DVE and DMA'd out once.
  * Diagonal terms: the PE accumulator rounds toward zero, so sim_ii from a
    host-side float64 emulation is biased low by ~2^-24*D relative -- enough
    to shift the loss by 2%.  Instead the bf16 exp of sim_ii is extracted
    from the e-tile on device (one masked scalar_tensor_tensor with
    accum_out per diagonal candidate) and the host uses x_i = log(e_ii),
    which cancels structurally in log(denom_i) - x_i.
  * Host: float64 combination of row/col partials, dead-row corrections, log,
    mean.
"""

import math

import numpy as np

TEMPERATURE = 0.07
DROP_P = 0.5
EPS = 1e-8
NCORES = 8
KP = 128     # partition size
PANEL = 512  # max matmul moving free dim (one PSUM bank of fp32)
FP8_SCALE = 16.0
CSG = 4      # colsum jobs packed per PSUM tile (column tiling)
CSLAG = 4    # extra jobs of lag before a colsum pack is emitted

_CACHE = {}


# --------------------------------------------------------------------------
# host-side preparation
# --------------------------------------------------------------------------

def _host_prep(features, labels, aug_indices):
    """Mirror the reference's prologue op-for-op on the default jax backend so
    the dropout PRNG bits and fn values match the graded reference exactly."""
    import jax
    import jax.numpy as jnp

    features = jnp.asarray(np.asarray(features))
    labels_np = np.asarray(labels)
    aug_np = np.asarray(aug_indices)

    pert = features[jnp.asarray(aug_np)]
    keep = jax.random.bernoulli(jax.random.key(42), 1.0 - DROP_P, pert.shape)
    pert = jnp.where(keep, pert * 2.0, jnp.zeros((), dtype=pert.dtype))
    feats = jnp.concatenate([features, pert], axis=0)

    norms = jnp.sqrt(jnp.sum(feats * feats, axis=1, keepdims=True))
    fn = np.asarray(feats / jnp.maximum(norms, EPS)).astype(np.float32)
    labs = np.concatenate([labels_np, labels_np[aug_np]], axis=0)

    perm = np.argsort(labs, kind="stable")
    fn_sorted = np.ascontiguousarray(fn[perm])
    labs_sorted = labs[perm]
    return fn, labs, perm, fn_sorted, labs_sorted


class _Plan:
    """Compile-time structure shared by program builder, simulator, finisher.

    Per class c (counts in sorted-label order):
      RT[c]  global 128-row tiles;  R[c] = ceil(RT/8) per-core row slots
      P[c]   column panels;  w(c,p) widths (last panel exact)
    Core k's row slot (c, j) holds physical tile t = k + 8*j (dead if t>=RT).
    Structural jobs per class: {(p, j): j <= p//2, j < R[c]} — on core k the
    job is *counted* iff t real and p >= t//4 (upper-or-diagonal).

    Panels are organized into GROUPS: [(0,), (1,2), (3,4), ...] — pairs of
    full (512-wide) panels, everything else single.  Acts (activation units):
      pair group (p1,p2): paired act for each j <= (p1-1)//2  (jobs on both
        panels), plus the diagonal single act (p2, j=p2//2) if j < R.
      single group (p,): one act per j <= p//2.
    """

    def __init__(self, n2, d, class_counts):
        assert d % 256 == 0
        self.n2 = n2
        self.d = d
        self.k2t = d // 256   # double-row contraction steps per job
        self.counts = list(class_counts)
        self.ncls = len(self.counts)
        self.RT = [math.ceil(c / KP) for c in self.counts]
        self.R = [math.ceil(rt / NCORES) for rt in self.RT]
        self.P = [math.ceil(c / PANEL) for c in self.counts]
        self.Wreal = [c - (p - 1) * PANEL for c, p in zip(self.counts, self.P)]
        self.W = [w + (w & 1) for w in self.Wreal]
        self.S = [r * KP for r in self.R]
        self.row_slots = sum(self.S)
        self.col_slots = sum(p * PANEL for p in self.P)
        self.npanels = sum(self.P)
        self.nrt = sum(self.R)
        self.cls_row_off = np.cumsum([0] + self.counts).tolist()
        self.slot_off = np.cumsum([0] + self.S).tolist()
        self.panel_off = np.cumsum([0] + [p * PANEL for p in self.P]).tolist()
        self.panel_idx = {}
        pi = 0
        for c in range(self.ncls):
            for p in range(self.P[c]):
                self.panel_idx[(c, p)] = pi
                pi += 1

        def width(c, p):
            return PANEL if p < self.P[c] - 1 else self.W[c]

        self.width = width

        # panel groups per class: (0,), then (odd, odd+1) pairs of full
        # panels, leftovers single
        groups = []
        for c in range(self.ncls):
            p = 0
            while p < self.P[c]:
                if (p % 2 == 1 and p + 1 < self.P[c]
                        and width(c, p) == PANEL
                        and width(c, p + 1) == PANEL):
                    groups.append((c, (p, p + 1)))
                    p += 2
                else:
                    groups.append((c, (p,)))
                    p += 1

        def njobs_of(c, p):
            return min(p // 2 + 1, self.R[c])

        def group_jobs(c, ps):
            return sum(njobs_of(c, p) for p in ps)

        groups.sort(key=lambda g: -group_jobs(*g))
        inter, lo, hi = [], 0, len(groups) - 1
        while lo <= hi:
            inter.append(groups[lo]); lo += 1
            if lo <= hi:
                inter.append(groups[hi]); hi -= 1
        self.group_seq = inter

        # acts and jobs, in emission order
        self.acts = []   # (c, j, (p,...), (jid,...))
        self.jobs = []   # (c, p, j, w); index = jid
        self.job_id = {}
        for c, ps in self.group_seq:
            if len(ps) == 2:
                p1, p2 = ps
                for j in range(min((p1 - 1) // 2 + 1, self.R[c])):
                    j1 = self._add_job(c, p1, j)
                    j2 = self._add_job(c, p2, j)
                    self.acts.append((c, j, (p1, p2), (j1, j2)))
                jd = p2 // 2
                if jd < self.R[c]:
                    jid = self._add_job(c, p2, jd)
                    self.acts.append((c, jd, (p2,), (jid,)))
            else:
                p = ps[0]
                for j in range(njobs_of(c, p)):
                    jid = self._add_job(c, p, j)
                    self.acts.append((c, j, (p,), (jid,)))
        self.njobs = len(self.jobs)
        self.nacts = len(self.acts)
        self.ngroups = math.ceil(self.njobs / CSG)
        # acts of (c, j) with min panel and fake-col count
        self.acts_of = {}
        for aid, (c, j, ps, jids) in enumerate(self.acts):
            fake = sum(self.width(c, p) - (self.Wreal[c]
                       if p == self.P[c] - 1 else PANEL) for p in ps)
            self.acts_of.setdefault((c, j), []).append(
                (aid, min(ps), fake))

    def _add_job(self, c, p, j):
        jid = len(self.jobs)
        self.jobs.append((c, p, j, self.width(c, p)))
        self.job_id[(c, p, j)] = jid
        return jid

    def rowtile_index(self, c, j):
        return sum(self.R[cc] for cc in range(c)) + j

    def phys_tile(self, core, j):
        return core + NCORES * j

    def realrows(self, c, t):
        return int(min(max(self.counts[c] - KP * t, 0), KP))


def _build_host_arrays(plan, fn_sorted):
    """fp8 cols tensor (shared) and per-core fp8 lhsT tensors.

    cols layout: [KP, npanels, k2t, 2, PANEL] — per partition, one panel is
    a contiguous (k2t*2*PANEL)-byte run, so a panel group loads in one DMA.
    element [p, pnl, k, i, c] = fnT_q[256k + 128i + p, panel_col(pnl) + c]

    lhsT layout: [KP, k2t, 2, row_slots]
    element [p, k, i, m] = fnT_q[256k + 128i + p, row_of_slot(m)]
    """
    import ml_dtypes

    k2t = plan.k2t
    fn_q = (fn_sorted * np.float32(FP8_SCALE)).astype(ml_dtypes.float8_e4m3)
    fnT_q = np.ascontiguousarray(fn_q.T)          # [D, n2] fp8

    colsD = np.zeros((plan.d, plan.col_slots), dtype=ml_dtypes.float8_e4m3)
    for c in range(plan.ncls):
        n = plan.counts[c]
        colsD[:, plan.panel_off[c]: plan.panel_off[c] + n] = (
            fnT_q[:, plan.cls_row_off[c]: plan.cls_row_off[c] + n])
    cols = np.ascontiguousarray(
        colsD.reshape(k2t, 2, KP, plan.npanels, PANEL)
        .transpose(2, 3, 0, 1, 4))

    lhsTs = []
    for core in range(NCORES):
        lt = np.zeros((plan.d, plan.row_slots), dtype=ml_dtypes.float8_e4m3)
        for c in range(plan.ncls):
            for j in range(plan.R[c]):
                t = plan.phys_tile(core, j)
                if t >= plan.RT[c]:
                    continue
                nreal = plan.realrows(c, t)
                src = fnT_q[:, plan.cls_row_off[c] + KP * t:
                            plan.cls_row_off[c] + KP * t + nreal]
                off = plan.slot_off[c] + j * KP
                lt[:, off: off + nreal] = src
        lhsTs.append(np.ascontiguousarray(
            lt.reshape(k2t, 2, KP, plan.row_slots).transpose(2, 0, 1, 3)))
    return cols, lhsTs


# --------------------------------------------------------------------------
# bass program
# --------------------------------------------------------------------------

def _build_program(plan, reps=1):
    import os
    probe = os.environ.get("KPROBE", "")
    import concourse.bacc as bacc
    import concourse.tile as tile
    import concourse.mybir as mybir

    f32 = mybir.dt.float32
    bf16 = mybir.dt.bfloat16
    f8 = mybir.dt.float8e4
    DR = mybir.MatmulPerfMode.DoubleRow
    scale_dev = float(1.0 / (np.float32(TEMPERATURE) * np.float32(FP8_SCALE)
                             * np.float32(FP8_SCALE)))
    k2t = plan.k2t

    nc = bacc.Bacc("TRN2", target_bir_lowering=False, debug=False)
    lhsT_d = nc.dram_tensor("lhsT", [KP, k2t, 2, plan.row_slots], f8,
                            kind="ExternalInput")
    cols_d = nc.dram_tensor("cols", [KP, plan.npanels, k2t, 2, PANEL], f8,
                            kind="ExternalInput")
    dmask_d = nc.dram_tensor("dmask", [KP, PANEL], bf16, kind="ExternalInput")
    part_d = nc.dram_tensor("partials", [KP, plan.nacts], f32,
                            kind="ExternalOutput")
    csum_d = nc.dram_tensor("csum", [CSG, plan.ngroups * PANEL], f32,
                            kind="ExternalOutput")
    diag_d = nc.dram_tensor("diag", [KP, plan.nrt, 2], f32,
                            kind="ExternalOutput")

    with tile.TileContext(nc) as tc:
        with (
            tc.tile_pool(name="persist", bufs=1) as persist,
            tc.tile_pool(name="panels", bufs=4) as panels,
            tc.tile_pool(name="work", bufs=4) as work,
            tc.tile_pool(name="psum", bufs=3, space="PSUM") as psum_main,
            tc.tile_pool(name="psumc", bufs=2, space="PSUM") as psum_cs,
        ):
            lhsT = persist.tile([KP, k2t, 2, plan.row_slots], f8)
            nc.sync.dma_start(out=lhsT, in_=lhsT_d[:])
            ones = persist.tile([KP, 1], bf16)
            nc.vector.memset(ones, 1.0)
            dmask = persist.tile([KP, PANEL], bf16)
            nc.sync.dma_start(out=dmask, in_=dmask_d[:])
            partials = persist.tile([KP, plan.nacts], f32)
            stage = persist.tile([KP, plan.ngroups * PANEL], f32)
            diag = persist.tile([KP, plan.nrt, 2], f32)

            def emit_body():
                # Colsum matmuls depend on ScalarE exp output; a pack of CSG
                # is emitted only once CSLAG further jobs' mains are out, so
                # the in-order PE never stalls on ScalarE.  The CSG matmuls
                # issue back-to-back into one PSUM tile via column tiling
                # (concurrent in the PE array), drained to `stage` by DVE.
                pending = []          # (e_ap, w, jid)

                def flush_group():
                    if probe == "nocsum":
                        del pending[:CSG]
                        return
                    g0 = pending[0][2] // CSG
                    pcs = psum_cs.tile([KP, PANEL], f32, name="pcs")
                    for e_, w_, jid_ in pending[:CSG]:
                        r = jid_ % CSG
                        nc.tensor.matmul(
                            pcs[32 * r:32 * r + 1, :w_],
                            ones, e_[:, :w_], start=True, stop=True,
                            tile_position=(0, 32 * r))
                    nc.vector.tensor_copy(
                        stage[0:CSG * 32 - 31, g0 * PANEL:(g0 + 1) * PANEL],
                        pcs[0:CSG * 32 - 31, :])
                    del pending[:CSG]

                last_panel = [None]
                for c, ps in plan.group_seq:
                    glen = len(ps)
                    if probe == "nopanels" and last_panel[0] is not None:
                        panel = last_panel[0]
                    else:
                        panel = panels.tile([KP, 2, k2t, 2, PANEL], f8,
                                            name="panel")
                        last_panel[0] = panel
                        pidx = plan.panel_idx[(c, ps[0])]
                        nc.sync.dma_start(
                            out=panel[:, 0:glen],
                            in_=cols_d[:, pidx:pidx + glen])
                    # acts of this group, in plan order
                    for aid, (ac, aj, aps, ajids) in enumerate(plan.acts):
                        if ac != c or tuple(sorted(set(aps) - set(ps))):
                            continue
                        j = aj
                        off = plan.slot_off[c] + j * KP
                        pst = psum_main.tile([KP, 2, PANEL], f32, name="ps")
                        for pi_local, p in enumerate(aps):
                            half = ps.index(p)
                            w = plan.width(c, p)
                            for k in range(k2t):
                                nc.tensor.matmul(
                                    pst[:, pi_local, :w],
                                    lhsT[:, k, :, off:off + KP],
                                    panel[:, half, k, :, :w],
                                    start=(k == 0), stop=(k == k2t - 1),
                                    perf_mode=DR)
                        if probe == "noact":
                            continue
                        e = work.tile([KP, 2, PANEL], bf16, tag="etile",
                                      name="e", bufs=10)
                        if len(aps) == 2:
                            nc.scalar.activation(
                                out=e[:, :, :], in_=pst[:, :, :],
                                func=mybir.ActivationFunctionType.Exp,
                                scale=scale_dev,
                                accum_out=partials[:, aid:aid + 1])
                        else:
                            w = plan.width(c, aps[0])
                            nc.scalar.activation(
                                out=e[:, 0, :w], in_=pst[:, 0, :w],
                                func=mybir.ActivationFunctionType.Exp,
                                scale=scale_dev,
                                accum_out=partials[:, aid:aid + 1])
                        for pi_local, (p, jid) in enumerate(zip(aps, ajids)):
                            w = plan.width(c, p)
                            if p in (2 * j, 2 * j + 1) and probe != "nodiag":
                                junk = work.tile([KP, PANEL], bf16, tag="dj",
                                                 name="dj", bufs=2)
                                t_idx = plan.rowtile_index(c, j)
                                parity = p - 2 * j
                                nc.vector.scalar_tensor_tensor(
                                    out=junk[:, :w], in0=e[:, pi_local, :w],
                                    scalar=1.0, in1=dmask[:, :w],
                                    op0=mybir.AluOpType.mult,
                                    op1=mybir.AluOpType.mult,
                                    accum_out=diag[:, t_idx,
                                                   parity:parity + 1])
                            pending.append((e[:, pi_local], w, jid))
                        while len(pending) >= CSG + CSLAG:
                            flush_group()
                while pending:
                    flush_group()

            if reps > 1:
                with tc.For_i(0, reps, 1):
                    emit_body()
            else:
                emit_body()

            if probe != "noact":
                nc.sync.dma_start(out=part_d[:], in_=partials)
            if probe not in ("noact", "nocsum"):
                nc.sync.dma_start(out=csum_d[:],
                                  in_=stage[0:CSG * 32 - 31:32, :])
            if probe not in ("noact", "nodiag"):
                nc.sync.dma_start(out=diag_d[:], in_=diag)
    nc.compile()
    return nc


# --------------------------------------------------------------------------
# numpy simulation of the device outputs (for logic validation)
# --------------------------------------------------------------------------

def _simulate_device(plan, cols, lhsTs):
    import ml_dtypes

    scale_dev = np.float32(1.0 / (np.float32(TEMPERATURE)
                                  * np.float32(FP8_SCALE) ** 2))
    results = []
    k2t = plan.k2t
    colsf = (cols.transpose(2, 3, 0, 1, 4)
             .reshape(plan.d, plan.npanels * PANEL).astype(np.float32))
    for core in range(NCORES):
        lt = (lhsTs[core].transpose(1, 2, 0, 3)
              .reshape(plan.d, plan.row_slots).astype(np.float32))
        partials = np.zeros((KP, plan.nacts), dtype=np.float32)
        csum = np.zeros((CSG, plan.ngroups * PANEL), dtype=np.float32)
        diag = np.zeros((KP, plan.nrt, 2), dtype=np.float32)
        doff = (core % 4) * KP
        for aid, (c, j, aps, ajids) in enumerate(plan.acts):
            off = plan.slot_off[c] + j * KP
            for p, jid in zip(aps, ajids):
                w = plan.width(c, p)
                c0 = plan.panel_idx[(c, p)] * PANEL
                s = (lt[:, off:off + KP].T @ colsf[:, c0:c0 + w]
                     ).astype(np.float32)
                e = np.exp((s * scale_dev).astype(np.float32),
                           dtype=np.float32)
                partials[:, aid] += e.sum(axis=1, dtype=np.float32)
                ebf = e.astype(ml_dtypes.bfloat16).astype(np.float32)
                g, r = jid // CSG, jid % CSG
                csum[r, g * PANEL:g * PANEL + w] = ebf.sum(axis=0,
                                                           dtype=np.float32)
                if p in (2 * j, 2 * j + 1):
                    d = np.zeros(KP, dtype=np.float32)
                    n = max(0, min(KP, w - doff))
                    d[:n] = ebf[np.arange(n), doff + np.arange(n)]
                    diag[:, plan.rowtile_index(c, j), p - 2 * j] = d
        results.append({"partials": partials, "csum": csum, "diag": diag})
    return results


# --------------------------------------------------------------------------
# host-side finish
# --------------------------------------------------------------------------

def _finish(plan, results):
    """Combine per-core device outputs into the scalar loss (float64).

    Row i (class c, class-row g = 128*t + i, owner core k = t%8, j = t//8):
      denom_g = sum over acts of (c,j) with min_p >= t//4 of
                    partials[i, act] - fake(act)                       (rows)
              + sum over tiles t' with t'//4 < p_g of
                    csum[job(c, p_g, j')][g - 512*p_g] - dead(t')      (cols)
      loss_g  = log(denom_g) - log(diag_e[i])
    """
    total = 0.0
    nrows = 0
    for c in range(plan.ncls):
        cnt = plan.counts[c]
        denom = np.zeros(cnt, dtype=np.float64)
        x = np.zeros(cnt, dtype=np.float64)
        for core in range(NCORES):
            partials = results[core]["partials"].astype(np.float64)
            csum = results[core]["csum"].astype(np.float64)
            diag = results[core]["diag"].astype(np.float64)
            for j in range(plan.R[c]):
                t = plan.phys_tile(core, j)
                if t >= plan.RT[c]:
                    continue
                m = plan.realrows(c, t)
                rows = slice(KP * t, KP * t + m)
                # row-sum contributions: acts with min_p >= t//4
                for aid, minp, fake in plan.acts_of[(c, j)]:
                    if minp < t // 4:
                        continue
                    denom[rows] += partials[:m, aid] - fake
                # col-sum contributions: strictly-upper jobs (p > t//4)
                for p in range(t // 4 + 1, plan.P[c]):
                    jid = plan.job_id.get((c, p, j))
                    if jid is None:
                        continue
                    wr = min(plan.jobs[jid][3], plan.counts[c] - PANEL * p)
                    cols_sl = slice(PANEL * p, PANEL * p + wr)
                    g, r = jid // CSG, jid % CSG
                    dead = KP - m
                    denom[cols_sl] += (
                        csum[r, g * PANEL:g * PANEL + wr] - dead)
                # diagonal: log of the bf16 exp of sim_ii, extracted from
                # the parity (core//4) candidate job's e-tile
                x[rows] = np.log(
                    diag[:m, plan.rowtile_index(c, j), core // 4])
        total += float(np.sum(np.log(denom) - x))
        nrows += cnt
    assert nrows == plan.n2, (nrows, plan.n2)
    return np.float32(total / nrows)


# --------------------------------------------------------------------------
# entry point
# --------------------------------------------------------------------------

def _get_compiled(plan, reps=1):
    key = (plan.n2, plan.d, tuple(plan.counts), reps)
    if key not in _CACHE:
        _CACHE[key] = _build_program(plan, reps=reps)
    return _CACHE[key]


def _prepare(inputs):
    features = np.asarray(inputs["features"])
    labels = np.asarray(inputs["labels"])
    aug_indices = np.asarray(inputs["aug_indices"])

    fn, labs, perm, fn_sorted, labs_sorted = _host_prep(
        features, labels, aug_indices)
    n2, d = fn_sorted.shape
    classes, counts = np.unique(labs_sorted, return_counts=True)
    plan = _Plan(n2, d, counts.tolist())
    cols, lhsTs = _build_host_arrays(plan, fn_sorted)
    import ml_dtypes
    in_maps = []
    for core in range(NCORES):
        dmask = np.zeros((KP, PANEL), dtype=ml_dtypes.bfloat16)
        off = (core % 4) * KP
        dmask[np.arange(KP), off + np.arange(KP)] = 1.0
        in_maps.append({"lhsT": lhsTs[core], "cols": cols, "dmask": dmask})
    return plan, cols, lhsTs, in_maps


def kernel(simulate=False, **inputs):
    plan, cols, lhsTs, in_maps = _prepare(inputs)

    if simulate:
        results = _simulate_device(plan, cols, lhsTs)
    else:
        from concourse.bass_utils import run_bass_kernel_spmd

        nc = _get_compiled(plan)
        results = run_bass_kernel_spmd(nc, in_maps,
                                       core_ids=list(range(NCORES))).results

    return np.asarray(_finish(plan, results), dtype=np.float32)


# --------------------------------------------------------------------------
# timing harness (mirrors bass2jax.run_bass_via_pjrt's multi-core path but
# keeps the big inputs device-resident so repeated calls time the NEFF)
# --------------------------------------------------------------------------

def _make_sharded(nc, n_cores):
    import jax
    import concourse.mybir as mybir
    from jax.sharding import Mesh, PartitionSpec
    from jax.experimental.shard_map import shard_map
    from concourse.bass2jax import (_bass_exec_p, install_neuronx_cc_hook,
                                    partition_id_tensor)

    install_neuronx_cc_hook()
    partition_name = (nc.partition_id_tensor.name
                      if nc.partition_id_tensor else None)
    in_names, out_names, out_avals, zero_outs = [], [], [], []
    for alloc in nc.m.functions[0].allocations:
        if not isinstance(alloc, mybir.MemoryLocationSet):
            continue
        name = alloc.memorylocations[0].name
        if alloc.kind == "ExternalInput":
            if name != partition_name:
                in_names.append(name)
        elif alloc.kind == "ExternalOutput":
            out_names.append(name)
            shape = tuple(alloc.tensor_shape)
            dtype = mybir.dt.np(alloc.dtype)
            out_avals.append(jax.core.ShapedArray(shape, dtype))
            zero_outs.append(np.zeros(shape, dtype))
    n_params = len(in_names)
    all_names = in_names + out_names
    if partition_name is not None:
        all_names.append(partition_name)

    def _body(*args):
        operands = list(args)
        if partition_name is not None:
            operands.append(partition_id_tensor())
        outs = _bass_exec_p.bind(
            *operands,
            out_avals=tuple(out_avals),
            in_names=tuple(all_names),
            out_names=tuple(out_names),
            lowering_input_output_aliases=(),
            sim_require_finite=True,
            sim_require_nnan=True,
            nc=nc,
        )
        return tuple(outs)

    devices = jax.devices()[:n_cores]
    mesh = Mesh(np.asarray(devices), ("core",))
    in_specs = (PartitionSpec("core"),) * (n_params + len(out_names))
    out_specs = (PartitionSpec("core"),) * len(out_names)
    donate = tuple(range(n_params, n_params + len(out_names)))
    sharded = jax.jit(
        shard_map(_body, mesh=mesh, in_specs=in_specs, out_specs=out_specs,
                  check_rep=False),
        donate_argnums=donate, keep_unused=True)
    return sharded, in_names, out_names, out_avals, zero_outs, mesh


def _make_runner(nc, in_maps):
    import jax
    from jax.sharding import NamedSharding, PartitionSpec

    sharded, in_names, out_names, out_avals, zero_outs, mesh = _make_sharded(
        nc, NCORES)
    concat_in = [np.concatenate([in_maps[c][n] for c in range(NCORES)], axis=0)
                 for n in in_names]
    sharding = NamedSharding(mesh, PartitionSpec("core"))
    dev_in = [jax.device_put(a, sharding) for a in concat_in]

    def run():
        import time
        zs = [jax.device_put(
            np.zeros((NCORES * z.shape[0], *z.shape[1:]), z.dtype), sharding)
            for z in zero_outs]
        jax.block_until_ready(zs)
        t0 = time.perf_counter()
        out = sharded(*dev_in, *zs)
        jax.block_until_ready(out)
        return time.perf_counter() - t0

    run()  # warmup (compile + first exec)
    return run


def benchmark(loop_reps=1025, pairs=12, **inputs):
    """Per-iteration kernel time, cancelling the ~70ms axon dispatch floor:
    interleave timings of a 1-rep NEFF and a `loop_reps`-rep NEFF (HW loop)
    and difference the minima."""
    plan, cols, lhsTs, in_maps = _prepare(inputs)
    run1 = _make_runner(_get_compiled(plan, reps=1), in_maps)
    runR = _make_runner(_get_compiled(plan, reps=loop_reps), in_maps)

    t1s, tRs = [], []
    for _ in range(pairs):
        t1s.append(run1())
        tRs.append(runR())
    m1, mR = min(t1s), min(tRs)
    per_iter = (mR - m1) / (loop_reps - 1)
    print(f"  [bench] min T(1)={m1*1e3:.2f}ms  min T({loop_reps})={mR*1e3:.2f}ms")
    return per_iter * 1e9
